# revision 11
# baseline (speedup 1.0000x reference)
"""Trainium2 Bass kernel for batched FK chain with MLP joint correction.

Math: reference computes, per batch row,
    corr = tanh MLP(joints);  theta = joints + corr
    M_j = DH(alpha_j, a_j, d_j, theta_j + off_j);  ee = M_0 @ ... @ M_6
    out = ee[:3, 3]
Key factorization: M_j = A_j @ Rz(th_j) with A_j constant (from fk_params only),
and col 3 of M_6 is constant, so
    p = t_6;  for j = 5..0:  p = A_j @ Rz(th_j) @ p      (3-vector affine chain)

Device pipeline per core (Bc = 32768 rows):
  - feature-major x image [128, 2048] (16 batch groups packed on partitions)
  - 3 MLP layers as fp16 block-pattern matmuls on PE (8 rows/cycle)
  - tanh on ACT with per-partition bias (b1' = b1 - W1 @ off folds the DH offset)
  - PE transpose-mode matmuls accumulate x.T + corr.T into PSUM (free add)
  - sin/cos via ACT Sin (cos = sin(x + pi/2)); both live in one table set
  - chain of 6 (z-rot + const affine) steps on DVE/GPSIMD/ACT, fp16 planes
"""

import os
import numpy as np

import concourse.bass as bass
import concourse.tile as tile
from concourse import bacc, mybir
from concourse import bass_utils

N_CORES = 8
B = 262144
BC = B // N_CORES            # 32768 rows per core
PLANE = BC // 128            # 256  (plane free size)
NCH = 16                     # transpose chunks of 128 cols

F16 = mybir.dt.float16
F32 = mybir.dt.float32
AF = mybir.ActivationFunctionType
OP = mybir.AluOpType

# ---- constants blob column map -------------------------------------------
C_BIAS1, C_BIAS2, C_BIAS3, C_HALFPI = 0, 1, 2, 3
# step-5 (first chain step) compound scalars
C_S5U1M, C_S5U1A, C_S5XM = 4, 5, 6
C_S5U3M, C_S5U3A, C_S5YM = 7, 8, 9
C_S5U5M, C_S5U5A, C_S5ZM = 10, 11, 12
# generic steps j=4..0: 5 scalars each starting at col 16: a, ca, sa, dsa, cad
def _CJ(j, k):
    return 16 + 5 * j + k
NCONST = 48


def _build_host_data(inputs):
    joints = np.asarray(inputs["joints"], np.float32)
    fk = np.asarray(inputs["fk_params"], np.float32)
    W1 = np.asarray(inputs["W1"], np.float32)
    b1 = np.asarray(inputs["b1"], np.float32)
    W2 = np.asarray(inputs["W2"], np.float32)
    b2 = np.asarray(inputs["b2"], np.float32)
    W3 = np.asarray(inputs["W3"], np.float32)
    b3 = np.asarray(inputs["b3"], np.float32)

    alpha, a, d, off = fk[:, 0], fk[:, 1], fk[:, 2], fk[:, 3]
    ca, sa = np.cos(alpha), np.sin(alpha)
    b1p = b1 - W1 @ off
    x_off = joints + off[None, :]          # [B, 7] fp32
    # exact host range-reduction for the angle path (Sin on ACT needs [-pi, pi];
    # device uses half-angle identities so th = x_red + corr stays in range)
    x_red = (np.remainder(x_off + np.pi, 2 * np.pi) - np.pi).astype(np.float32)

    # --- per-core feature-major images ---
    # batch row b = 256*p + 16*c + 8*h + g; partition q = 64*h + 8*k + g
    # x_img[q, 128*c + p] = x_off[b, k]
    def mkimg(src, dtype):
        out = []
        for core in range(N_CORES):
            jc = src[core * BC:(core + 1) * BC]            # [32768, 7]
            arr = jc.reshape(128, 16, 2, 8, 7)             # [p, c, h, g, k]
            arr = arr.transpose(2, 4, 3, 1, 0)             # [h, k, g, c, p]
            img = np.zeros((2, 8, 8, 16, 128), np.float32)
            img[:, :7] = arr
            out.append(np.ascontiguousarray(img.reshape(128, 2048)).astype(dtype))
        return out
    imgs32 = mkimg(x_red, np.float32)
    imgs16 = mkimg(x_off, np.float16)

    # --- block-pattern weights (fp16), packed into one [128, 304] blob ---
    blob = np.zeros((128, 304), np.float16)
    # L1: lhsT1[64h+8k+g, 15g+j] = W1[j, k]  (cols 0:120)
    for h in (0, 1):
        for k in range(7):
            for g in range(8):
                blob[64 * h + 8 * k + g, 15 * g:15 * g + 15] = W1[:, k]
    # L2: lhsT2[15g+i, 15g+j] = W2[j, i]  (cols 120:240)
    for g in range(8):
        blob[15 * g:15 * g + 15, 120 + 15 * g:120 + 15 * g + 15] = W2.T
    # L3: lhsT3[15g+i, 8k+g] = W3[k, i]  (cols 240:304)
    for g in range(8):
        for k in range(7):
            blob[15 * g:15 * g + 15, 240 + 8 * k + g] = W3[k, :]

    # --- constants blob [128, NCONST] fp32 ---
    consts = np.zeros((128, NCONST), np.float32)
    for g in range(8):
        for j in range(15):
            consts[15 * g + j, C_BIAS1] = b1p[j]
            consts[15 * g + j, C_BIAS2] = b2[j]
    for h in (0, 1):
        for k in range(7):
            for g in range(8):
                consts[64 * h + 8 * k + g, C_BIAS3] = b3[k]
    consts[:, C_HALFPI] = np.pi / 2

    t6 = np.array([a[6], -d[6] * sa[6], ca[6] * d[6]], np.float32)
    C1 = -sa[5] * t6[2] - d[5] * sa[5]
    C2 = ca[5] * t6[2] + ca[5] * d[5]
    consts[:, C_S5U1M] = a[6]
    consts[:, C_S5U1A] = a[5]
    consts[:, C_S5XM] = -t6[1]
    consts[:, C_S5U3M] = ca[5] * a[6]
    consts[:, C_S5U3A] = C1
    consts[:, C_S5YM] = ca[5] * t6[1]
    consts[:, C_S5U5M] = sa[5] * a[6]
    consts[:, C_S5U5A] = C2
    consts[:, C_S5ZM] = sa[5] * t6[1]
    for j in range(5):
        consts[:, _CJ(j, 0)] = a[j]
        consts[:, _CJ(j, 1)] = ca[j]
        consts[:, _CJ(j, 2)] = sa[j]
        consts[:, _CJ(j, 3)] = d[j] * sa[j]
        consts[:, _CJ(j, 4)] = ca[j] * d[j]

    id32 = np.ascontiguousarray(np.eye(128, dtype=np.float32))
    id16 = np.ascontiguousarray(np.eye(128, dtype=np.float16))
    return imgs32, imgs16, blob, consts, id32, id16


def _emit_program(nc):
    dx16 = nc.dram_tensor("x16", [128, 2048], F16, kind="ExternalInput")
    dximg = nc.dram_tensor("ximg", [128, 2048], F32, kind="ExternalInput")
    dlhs = nc.dram_tensor("lhs", [128, 304], F16, kind="ExternalInput")
    dconsts = nc.dram_tensor("consts", [128, NCONST], F32, kind="ExternalInput")
    did32 = nc.dram_tensor("id32", [128, 128], F32, kind="ExternalInput")
    did16 = nc.dram_tensor("id16", [128, 128], F16, kind="ExternalInput")
    dout = nc.dram_tensor("out", [128, 768], F32, kind="ExternalOutput")

    from contextlib import ExitStack
    with tile.TileContext(nc) as tc, ExitStack() as ctx:
        cp = ctx.enter_context(tc.tile_pool(name="persist", bufs=1))
        mlp_ps = ctx.enter_context(tc.tile_pool(name="mlpps", bufs=2, space="PSUM"))
        tp_ps = ctx.enter_context(tc.tile_pool(name="tpps", bufs=4, space="PSUM"))
        chp = ctx.enter_context(tc.tile_pool(name="chain", bufs=2))

        consts = cp.tile([128, NCONST], F32, tag="consts")
        lhs = cp.tile([128, 304], F16, tag="lhs")
        x16 = cp.tile([128, 2048], F16, tag="x16")
        ximg = cp.tile([128, 2048], F32, tag="ximg")
        id32 = cp.tile([128, 128], F32, tag="id32")
        id16 = cp.tile([128, 128], F16, tag="id16")
        h1 = cp.tile([128, 4096], F16, tag="h1")
        h2 = cp.tile([128, 4096], F16, tag="h2")
        corr = cp.tile([128, 2048], F32, tag="corr")
        CT = cp.tile([128, 2048], F16, tag="CT")
        STt = cp.tile([128, 2048], F16, tag="ST")
        S2 = cp.tile([128, 2048], F16, tag="S2")
        S4 = cp.tile([128, 2048], F16, tag="S4")
        SQ = cp.tile([128, 2048], F16, tag="SQ")
        pack = cp.tile([128, 768], F32, tag="pack")

        nc.sync.dma_start(consts[:], dconsts.ap())
        nc.sync.dma_start(lhs[:], dlhs.ap())
        nc.sync.dma_start(x16[:], dx16.ap())
        nc.sync.dma_start(id32[:], did32.ap())
        nc.sync.dma_start(id16[:], did16.ap())
        nc.sync.dma_start(ximg[:], dximg.ap())

        def cv(col, parts=128):
            return consts[0:parts, col:col + 1]

        # ---- L1 + L2: feature-major matmuls, tanh -> fp16 hidden ----
        for h in (0, 1):
            for half in (0, 1):
                ps = mlp_ps.tile([128, 1024], F32, tag="mlpps")
                for s in (0, 1):
                    n0 = 1024 * half + 512 * s
                    nc.tensor.matmul(
                        ps[0:120, 512 * s:512 * s + 512],
                        lhs[64 * h:64 * h + 64, 0:120],
                        x16[64 * h:64 * h + 64, n0:n0 + 512],
                        start=True, stop=True,
                        tile_position=(64 * h, 0),
                    )
                nc.scalar.activation(
                    h1[0:120, 2048 * h + 1024 * half:2048 * h + 1024 * half + 1024],
                    ps[0:120, :], AF.Tanh, bias=cv(C_BIAS1, 120))
        for h in (0, 1):
            for half in (0, 1):
                ps = mlp_ps.tile([128, 1024], F32, tag="mlpps")
                for s in (0, 1):
                    col = 2048 * h + 1024 * half + 512 * s
                    nc.tensor.matmul(
                        ps[0:120, 512 * s:512 * s + 512],
                        lhs[0:120, 120:240],
                        h1[0:120, col:col + 512],
                        start=True, stop=True)
                nc.scalar.activation(
                    h2[0:120, 2048 * h + 1024 * half:2048 * h + 1024 * half + 1024],
                    ps[0:120, :], AF.Tanh, bias=cv(C_BIAS2, 120))
        # ---- L3: both batch halves stacked on psum partitions ----
        for half in (0, 1):
            ps = mlp_ps.tile([128, 1024], F32, tag="mlpps")
            for h in (0, 1):
                for s in (0, 1):
                    col = 2048 * h + 1024 * half + 512 * s
                    nc.tensor.matmul(
                        ps[64 * h:64 * h + 64, 512 * s:512 * s + 512],
                        lhs[0:120, 240:304],
                        h2[0:120, col:col + 512],
                        start=True, stop=True,
                        tile_position=(0, 64 * h))
            nc.scalar.activation(
                corr[:, 1024 * half:1024 * half + 1024],
                ps[:, :], AF.Tanh, bias=cv(C_BIAS3))

        # ---- transpose chunks: psum = x.T + corr.T ; then sin/cos ----
        for cb in range(4):
            ps = tp_ps.tile([128, 512], F32, tag="tpps")
            for cl in range(4):
                c = 4 * cb + cl
                nc.tensor.matmul(
                    ps[:, 128 * cl:128 * cl + 128],
                    ximg[:, 128 * c:128 * c + 128], id32[:],
                    is_transpose=True, start=True, stop=False)
                nc.tensor.matmul(
                    ps[:, 128 * cl:128 * cl + 128],
                    corr[:, 128 * c:128 * c + 128], id32[:],
                    is_transpose=True, start=False, stop=True)
            # S2/S4 flat layout: ((k*16 + c)*2 + h)*8 + g  (planes contiguous per k)
            in_v = ps[:, :].rearrange("p (c h k g) -> p c h k g", c=4, h=2, k=8, g=8)
            s2_v = S2[:, :].rearrange("p (k c h g) -> p c h k g", k=8, c=16, h=2, g=8)
            s4_v = S4[:, :].rearrange("p (k c h g) -> p c h k g", k=8, c=16, h=2, g=8)
            nc.scalar.activation(s2_v[:, 4 * cb:4 * cb + 4], in_v,
                                 AF.Sin, bias=0.0, scale=0.5)
            nc.scalar.activation(s4_v[:, 4 * cb:4 * cb + 4], in_v,
                                 AF.Sin, bias=0.0, scale=0.25)
        # cos(th) = 1 - 2*sin^2(th/2);  sin(th) = 2*sin(th/2)*(1 - 2*sin^2(th/4))
        nc.gpsimd.tensor_tensor(SQ[:], S2[:], S2[:], OP.mult)
        nc.vector.tensor_scalar(CT[:], SQ[:], -2.0, 1.0, OP.mult, OP.add)
        nc.vector.tensor_tensor(SQ[:], S4[:], S4[:], OP.mult)
        nc.vector.tensor_scalar(S4[:], SQ[:], -2.0, 1.0, OP.mult, OP.add)
        nc.vector.scalar_tensor_tensor(STt[:], S2[:], 2.0, S4[:], OP.mult, OP.mult)

        # ---- chain ----
        def ctj(j):
            return CT[:, 256 * j:256 * j + 256]

        def stj(j):
            return STt[:, 256 * j:256 * j + 256]

        def ch(tag, dtype=F16):
            return chp.tile([128, 256], dtype, tag=tag, name=tag)

        # step 5 (v = A5 Rz(th5) t6), all from compound consts
        u1 = ch("u1")
        nc.vector.tensor_scalar(u1, ctj(5), cv(C_S5U1M), cv(C_S5U1A), OP.mult, OP.add)
        x = ch("x")
        nc.vector.scalar_tensor_tensor(x, stj(5), cv(C_S5XM), u1, OP.mult, OP.add)
        u3 = ch("u3")
        nc.scalar.activation(u3, stj(5), AF.Identity,
                             bias=cv(C_S5U3A), scale=cv(C_S5U3M))
        y = ch("y")
        nc.vector.scalar_tensor_tensor(y, ctj(5), cv(C_S5YM), u3, OP.mult, OP.add)
        u5 = ch("u5")
        nc.scalar.activation(u5, stj(5), AF.Identity,
                             bias=cv(C_S5U5A), scale=cv(C_S5U5M))
        z = ch("z")
        nc.vector.scalar_tensor_tensor(z, ctj(5), cv(C_S5ZM), u5, OP.mult, OP.add)

        # steps j = 4..0
        for j in (4, 3, 2, 1, 0):
            last = j == 0
            t1 = ch("t1")
            nc.vector.tensor_tensor(t1, x, ctj(j), OP.mult)
            t2 = ch("t2")
            nc.gpsimd.tensor_tensor(t2, y, stj(j), OP.mult)
            t3 = ch("t3")
            nc.gpsimd.tensor_tensor(t3, x, stj(j), OP.mult)
            t4 = ch("t4")
            nc.vector.tensor_tensor(t4, y, ctj(j), OP.mult)
            # q1 = z*sa + d*sa ; q2 = z*ca + ca*d   (ACT, idle after sincos)
            q1 = ch("q1")
            nc.scalar.activation(q1, z, AF.Identity,
                                 bias=cv(_CJ(j, 3)), scale=cv(_CJ(j, 2)))
            q2 = ch("q2")
            nc.scalar.activation(q2, z, AF.Identity,
                                 bias=cv(_CJ(j, 4)), scale=cv(_CJ(j, 1)))
            xn = pack[:, 0:256] if last else ch("x")
            nc.vector.scalar_tensor_tensor(xn, t1, cv(_CJ(j, 0)), t2,
                                           OP.add, OP.subtract)
            yr = ch("yr")
            nc.gpsimd.tensor_tensor(yr, t3, t4, OP.add)
            yn = pack[:, 256:512] if last else ch("y")
            nc.vector.scalar_tensor_tensor(yn, yr, cv(_CJ(j, 1)), q1,
                                           OP.mult, OP.subtract)
            zn = pack[:, 512:768] if last else ch("z")
            nc.vector.scalar_tensor_tensor(zn, yr, cv(_CJ(j, 2)), q2,
                                           OP.mult, OP.add)
            x, y, z = xn, yn, zn

        nc.sync.dma_start(dout.ap(), pack[:])

    return dout


_PROG_CACHE = {}


def _get_program():
    if "nc" in _PROG_CACHE:
        return _PROG_CACHE["nc"]
    nc = bacc.Bacc("TRN2", target_bir_lowering=False, debug=False,
                   enable_asserts=False)
    _emit_program(nc)

    # Force Tanh and Sin to resolve to the one table set containing both
    # (silu_and_others), so the kernel pays a single ACT table load.
    import concourse.bacc as bacc_mod
    from concourse.hw_specs import get_activation_tables
    orig_fn = bacc_mod.get_activation_tables
    tabs = get_activation_tables(nc.m.arch)
    trig = {AF.Tanh, AF.Sin}
    doctored = {
        name: (set(funcs) if name == "silu_and_others" else set(funcs) - trig)
        for name, funcs in tabs.items()
    }
    bacc_mod.get_activation_tables = lambda arch: doctored
    try:
        nc.compile()
    finally:
        bacc_mod.get_activation_tables = orig_fn

    _PROG_CACHE["nc"] = nc
    return nc


LAST_RESULTS = None  # BassKernelResults of the most recent run (for test.py)


def _host_in_maps(inputs):
    imgs32, imgs16, blob, consts, id32, id16 = _build_host_data(inputs)
    in_maps = []
    for core in range(N_CORES):
        in_maps.append({
            "x16": imgs16[core],
            "ximg": imgs32[core],
            "lhs": blob,
            "consts": consts,
            "id32": id32,
            "id16": id16,
        })
    return in_maps


def time_on_hw(inputs, k_lo=2, k_hi=18, iters=7):
    """Marginal per-NEFF time: run the kernel K times back-to-back inside one
    jit (device-resident inputs) and fit the slope between k_lo and k_hi."""
    import time as _time
    import jax
    from jax.sharding import Mesh, PartitionSpec, NamedSharding
    from jax.experimental.shard_map import shard_map
    from concourse import bass2jax

    bass2jax.install_neuronx_cc_hook()
    nc = _get_program()
    in_maps = _host_in_maps(inputs)

    partition_name = nc.partition_id_tensor.name if nc.partition_id_tensor else None
    in_names, out_names, out_avals = [], [], []
    for alloc in nc.m.functions[0].allocations:
        if not isinstance(alloc, mybir.MemoryLocationSet):
            continue
        name = alloc.memorylocations[0].name
        if alloc.kind == "ExternalInput":
            if name != partition_name:
                in_names.append(name)
        elif alloc.kind == "ExternalOutput":
            out_names.append(name)
            out_avals.append(jax.core.ShapedArray(
                tuple(alloc.tensor_shape), mybir.dt.np(alloc.dtype)))
    n_params = len(in_names)
    all_in_names = in_names + out_names
    if partition_name is not None:
        all_in_names.append(partition_name)

    devices = jax.devices()[:N_CORES]
    mesh = Mesh(np.asarray(devices), ("core",))

    def make_f(k):
        def _body(*args):
            operands = list(args)
            if partition_name is not None:
                operands.append(bass2jax.partition_id_tensor())
            outs = None
            for _ in range(k):
                outs = bass2jax._bass_exec_p.bind(
                    *operands,
                    out_avals=tuple(out_avals),
                    in_names=tuple(all_in_names),
                    out_names=tuple(out_names),
                    lowering_input_output_aliases=(),
                    sim_require_finite=True,
                    sim_require_nnan=True,
                    nc=nc,
                )
            return tuple(outs)
        specs = (PartitionSpec("core"),) * (len(in_names) + len(out_names))
        ospec = (PartitionSpec("core"),) * len(out_names)
        return jax.jit(shard_map(_body, mesh=mesh, in_specs=specs,
                                 out_specs=ospec, check_rep=False))

    sh = NamedSharding(mesh, PartitionSpec("core"))
    concat_in = [
        jax.device_put(
            np.concatenate([np.asarray(in_maps[c][n]) for c in range(N_CORES)], axis=0),
            sh)
        for n in in_names
    ]
    concat_zero = [
        jax.device_put(
            np.zeros((N_CORES * a.shape[0], *a.shape[1:]), a.dtype), sh)
        for a in out_avals
    ]

    results = {}
    for k in (k_lo, k_hi):
        f = make_f(k)
        out = f(*concat_in, *concat_zero)
        jax.block_until_ready(out)   # compile + warm
        best = float("inf")
        for _ in range(iters):
            t0 = _time.perf_counter()
            out = f(*concat_in, *concat_zero)
            jax.block_until_ready(out)
            best = min(best, _time.perf_counter() - t0)
        results[k] = best
    slope_ns = (results[k_hi] - results[k_lo]) / (k_hi - k_lo) * 1e9
    print(f"[hw timing] t({k_lo})={results[k_lo]*1e6:.0f}us "
          f"t({k_hi})={results[k_hi]*1e6:.0f}us -> {slope_ns:.0f} ns/exec")
    return slope_ns


def kernel(**inputs):
    global LAST_RESULTS
    nc = _get_program()
    in_maps = _host_in_maps(inputs)
    res = bass_utils.run_bass_kernel_spmd(nc, in_maps, core_ids=list(range(N_CORES)))
    LAST_RESULTS = res

    out = np.empty((B, 3), np.float32)
    for core in range(N_CORES):
        p = res.results[core]["out"]                   # [128, 768]
        # pack cols: [px(256) | py(256) | pz(256)], b_local = 256*p + m
        oc = p.reshape(128, 3, 256).transpose(0, 2, 1).reshape(BC, 3)
        out[core * BC:(core + 1) * BC] = oc
    return out
